# revision 7
# baseline (speedup 1.0000x reference)
"""Trainium2 Bass kernel for nn_LocationSlayerRandom (SLAYER two-branch spiking net).

Contract: kernel(**inputs) takes the FULL unsharded inputs
  spike_input [32,156,1,1,2048] f32, W1 [512,156], W2 [20,512],
  Wl1 [512,2048], Wl2 [20,512], perm [156] i32
and returns the FULL output [32,20,1,1,2204] f32.

Strategy (8 cores, data-parallel over batch, 4 samples/core), v2:

Branch 1 (per sample b):  u1 = W1 @ psp_t(si)  (psp is linear => commutes)
  - psp_t(si): DVE tensor_tensor_scan along t (fp8 spike input, bf16 out),
    split into t-halves for pipelining; scans chain via initial=prev last col.
  - bf16 -> fp8 casts (DVE tensor_copy, plus two early ones on ACT) feed
  - fc1 on PE in fp8 DoubleRow: the (128-ch, 28-ch-tail) contraction pair in
    ONE 256-row pass via a step-sliced [128,2,512] AP over a packed ps8 tile.
  - thresholds all on ACT as Sign(u1-10) in {-1,0,1} fp8; fc2 weights
    pre-scaled 0.5 and the affine 0.5*rowsum(W2) folded into a host-side
    time-varying threshold T2[o,t] (bf16).
  - fc2 on PE fp8 with 4 samples in the 4 PE column groups -> one
    [128,1024] PSUM per t-half; psp scan from PSUM (bf16 out); compare vs
    T2 (all-bf16 2x DVE); bf16 output DMA.

Branch 2: ul1 = psp_c'(Wl1 @ x_tp), x_tp host-gathered/transposed (sipT).
  - A1 m-blocks on PE (fp8 DoubleRow over t), interleaved with fc1 to fill
    threshold-paced PE gaps; c'-psp scan straight from PSUM with the
    reset-pattern multiplier; l1 threshold on ACT Sign (wl2 0.5-scaled,
    rowsum correction in T2b).
  - locationFc2 col-tiled over samples: psum [32b+o, c'], so its psp scan is
    a [128,156] scan (4x shorter than [20,624]) and needs no reset pattern.

Numerics: all heavy matmuls fp8 with fp32 accumulate; psp states bf16.
The only nonlinearity is the >=10 threshold; true layer-2 potentials sit
below 3.2 (branch 1) / 2.0 (branch 2) against a threshold of 10, so
near-threshold layer-1 bit flips from low-precision weights/activations
cannot flip any output bit.
"""

from contextlib import ExitStack

import numpy as np
import ml_dtypes

import concourse.bass as bass
import concourse.mybir as mybir
from concourse import bacc
from concourse import tile as tile_mod
from concourse.bass_utils import run_bass_kernel_spmd

F32 = mybir.dt.float32
BF16 = mybir.dt.bfloat16
FP8 = mybir.dt.float8e4
AL = mybir.AluOpType
AF = mybir.ActivationFunctionType
BF16_NP = ml_dtypes.bfloat16
FP8_NP = ml_dtypes.float8_e4m3

B, C_IN, T = 32, 156, 2048
HID, OUT_DIM = 512, 20
CP = 156                      # permuted taxel axis (branch-2 "time")
N_CORES = 8
B_PER = B // N_CORES          # 4 samples per core
ALPHA = float(np.exp(-1.0 / 10.0))
THETA = 10.0
NB2 = B_PER * CP              # 624, branch-2 packed free dim
KT = T // 128                 # 16 k-tiles over t
H = T // 2                    # 1024, t-half


def build_program(tc, outs, ins):
    nc = tc.nc
    out = outs["out"]
    DR = mybir.MatmulPerfMode.DoubleRow

    with ExitStack() as ctx:
        consts = ctx.enter_context(tc.tile_pool(name="consts", bufs=1))
        work = ctx.enter_context(tc.tile_pool(name="work", bufs=1))
        psum = ctx.enter_context(tc.tile_pool(name="psum", bufs=4, space="PSUM"))

        # ---------------- constant patterns (gpsimd; SBUF only) ----------
        alpha_t = consts.tile([128, T], F32, tag="alpha")
        nc.gpsimd.memset(alpha_t[:], ALPHA)
        pat624 = consts.tile([128, NB2], F32, tag="pat624")
        nc.gpsimd.memset(pat624[:], ALPHA)
        for j in range(B_PER):
            nc.gpsimd.memset(pat624[:, j * CP:j * CP + 1], 0.0)
        bias_m10 = consts.tile([128, 1], F32, tag="bm10")
        nc.gpsimd.memset(bias_m10[:], -THETA)
        act_warm = consts.tile([128, 1], F32, tag="actwarm")
        nc.scalar.activation(act_warm[:], bias_m10[:], AF.Sign,
                             bias=bias_m10[:])

        # ---------------- inputs (ordered for earliest need) -------------
        siB = consts.tile([128, T], FP8, tag="siB")
        nc.sync.dma_start(siB[:], ins["siB"][:])
        siA = consts.tile([128, B_PER * T], FP8, tag="siA")
        nc.sync.dma_start(siA[:, 0:H], ins["siA"][:, 0:H])
        nc.sync.dma_start(siA[:, H:T], ins["siA"][:, H:T])
        w1dr = consts.tile([128, B_PER * 2 * HID], FP8, tag="w1dr")
        nc.sync.dma_start(w1dr[:], ins["W1dr"][:])
        sip = consts.tile([128, KT * NB2], FP8, tag="sip")
        nc.sync.dma_start(sip[:], ins["sipT"][:])
        wl1 = consts.tile([128, KT * HID], FP8, tag="wl1")
        nc.sync.dma_start(wl1[:], ins["Wl1T"][:])
        for b in range(1, B_PER):
            nc.sync.dma_start(siA[:, b * T:(b + 1) * T],
                              ins["siA"][:, b * T:(b + 1) * T])
        t2_t = consts.tile([128, T], BF16, tag="t2")
        nc.sync.dma_start(t2_t[:], ins["T2"][:])
        w2p = consts.tile([128, 4 * 32], FP8, tag="w2p")
        nc.sync.dma_start(w2p[:], ins["W2pT"][:])
        wl2 = consts.tile([128, 4 * 32], BF16, tag="wl2")
        nc.sync.dma_start(wl2[:], ins["Wl2T"][:])
        t2b = consts.tile([128, CP], BF16, tag="t2b")
        nc.sync.dma_start(t2b[:], ins["T2b"][:])

        # ---------------- persistent work tiles --------------------------
        psA = work.tile([128, B_PER * T], BF16, tag="psA")
        psB = work.tile([128, T], BF16, tag="psB")
        ps8 = work.tile([128, 5 * T], FP8, tag="ps8")
        ps8_3d = ps8[:].rearrange("p (n t) -> p n t", n=5)
        w1_4d = w1dr[:].rearrange("p (b k o) -> p b k o", b=B_PER, k=2)
        wl1_3d = wl1[:].rearrange("p (k o) -> p k o", o=HID)
        sip_3d = sip[:].rearrange("p (k c) -> p k c", c=NB2)
        sg = [work.tile([128, 4 * T], FP8, tag=f"sg{b}", name=f"sg{b}")
              for b in range(B_PER)]
        sg3 = [s[:].rearrange("p (m t) -> p m t", m=4) for s in sg]
        ul1 = [work.tile([128, NB2], F32, tag=f"ul1{m}", name=f"ul1{m}")
               for m in range(4)]
        l1 = [work.tile([128, NB2], BF16, tag=f"l1{m}", name=f"l1{m}")
              for m in range(4)]
        vs = work.tile([128, T], BF16, tag="vs")
        o1 = work.tile([128, T], BF16, tag="o1")
        vs2 = work.tile([128, CP], BF16, tag="vs2")
        o2 = work.tile([128, CP], BF16, tag="o2")

        # ---------------- emission helpers -------------------------------
        def scanA(b, h):
            sl = slice(b * T + h * H, b * T + (h + 1) * H)
            init = 0.0 if h == 0 else psA[:, b * T + H - 1:b * T + H]
            nc.vector.tensor_tensor_scan(psA[:, sl], alpha_t[:, 0:H],
                                         siA[:, sl], init, AL.mult, AL.add)

        def scanB(h):
            sl = slice(h * H, (h + 1) * H)
            init = 0.0 if h == 0 else psB[:, H - 1:H]
            nc.vector.tensor_tensor_scan(psB[:, sl], alpha_t[:, 0:H],
                                         siB[:, sl], init, AL.mult, AL.add)

        def cast(block, src, h, engine):
            sl = slice(h * H, (h + 1) * H)
            dst = ps8[:, block * T + h * H:block * T + (h + 1) * H]
            if engine == "act":
                nc.scalar.activation(dst, src[:, sl], AF.Copy)
            else:
                nc.vector.tensor_copy(dst, src[:, sl])

        def castA(b, h, engine):
            sl = slice(b * T + h * H, b * T + (h + 1) * H)
            dst = ps8[:, b * T + h * H:b * T + (h + 1) * H]
            if engine == "act":
                nc.scalar.activation(dst, psA[:, sl], AF.Copy)
            elif engine == "gp":
                nc.gpsimd.tensor_copy(dst, psA[:, sl])
            else:
                nc.vector.tensor_copy(dst, psA[:, sl])

        fc1_psum = {}

        def fc1(b, h):
            # fp8 DoubleRow: (psA_b block, psB block) pair via step-sliced AP
            for m in range(4):
                pu = psum.tile([128, 1024], F32, tag="psum",
                               name=f"pu{b}{h}{m}")
                lhs = w1_4d[:, b, :, m * 128:(m + 1) * 128]
                for ch in range(2):
                    csl = slice(h * H + ch * 512, h * H + (ch + 1) * 512)
                    rhs = ps8_3d[:, b:5:(4 - b), csl]
                    nc.tensor.matmul(pu[:, ch * 512:(ch + 1) * 512], lhs, rhs,
                                     start=True, stop=True, perf_mode=DR)
                fc1_psum[(b, h, m)] = pu

        def thr(b, h):
            hs = slice(h * H, (h + 1) * H)
            for m in range(4):
                nc.scalar.activation(sg3[b][:, m, hs], fc1_psum[(b, h, m)][:],
                                     AF.Sign, bias=bias_m10[:])

        a1_psum = {}

        def a1_mm(m):
            pa = psum.tile([128, 1024], F32, tag="psum", name=f"pa{m}")
            a1 = pa[:, :NB2]
            msl = slice(m * 128, (m + 1) * 128)
            for ki in range(KT // 2):
                st, sp = (ki == 0), (ki == KT // 2 - 1)
                lhs = wl1_3d[:, 2 * ki:2 * ki + 2, msl]
                nc.tensor.matmul(a1[:, 0:512], lhs,
                                 sip_3d[:, 2 * ki:2 * ki + 2, 0:512],
                                 start=st, stop=sp, perf_mode=DR)
                nc.tensor.matmul(a1[:, 512:NB2], lhs,
                                 sip_3d[:, 2 * ki:2 * ki + 2, 512:NB2],
                                 start=st, stop=sp, perf_mode=DR)
            a1_psum[m] = a1

        def a1_scan(m):
            nc.vector.tensor_tensor_scan(ul1[m][:], pat624[:], a1_psum[m],
                                         0.0, AL.mult, AL.add)

        def a1_thr(m):
            nc.scalar.activation(l1[m][:], ul1[m][:], AF.Sign,
                                 bias=bias_m10[:])

        def fc2b1(h):
            pu2 = psum.tile([128, 1024], F32, tag="psum", name=f"pu2{h}")
            for b in range(B_PER):
                for k in range(4):
                    ksl = slice(k * 32, k * 32 + 32)
                    for ch in range(2):
                        csl = slice(h * H + ch * 512, h * H + (ch + 1) * 512)
                        nc.tensor.matmul(pu2[32 * b:32 * b + 32,
                                             ch * 512:(ch + 1) * 512],
                                         w2p[:, ksl], sg3[b][:, k, csl],
                                         start=(k == 0), stop=(k == 3),
                                         tile_position=(0, 32 * b),
                                         skip_group_check=True)
            return pu2

        def fc2b1_post(h, pu2):
            hs = slice(h * H, (h + 1) * H)
            init = 0.0 if h == 0 else vs[:, H - 1:H]
            nc.vector.tensor_tensor_scan(vs[:, hs], alpha_t[:, 0:H],
                                         pu2[:], init, AL.mult, AL.add)
            nc.vector.tensor_tensor(o1[:, hs], vs[:, hs], t2_t[:, hs],
                                    AL.is_ge)
            nc.sync.dma_start(
                out[:, :, hs].rearrange("b j t -> (b j) t"), o1[:, hs])

        # ================= schedule =================
        # ---- phase h0 ----
        scanB(0)
        scanA(0, 0)
        cast(4, psB, 0, "act")
        castA(0, 0, "dve")
        fc1(0, 0)
        thr(0, 0)
        scanA(1, 0)
        castA(1, 0, "gp")
        fc1(1, 0)
        thr(1, 0)
        a1_mm(0)
        scanA(2, 0)
        castA(2, 0, "gp")
        fc1(2, 0)
        thr(2, 0)
        a1_mm(1)
        scanA(3, 0)
        castA(3, 0, "dve")
        fc1(3, 0)
        thr(3, 0)
        a1_mm(2)
        pu2_h0 = fc2b1(0)

        # ---- phase h1 ----
        scanB(1)
        cast(4, psB, 1, "act")
        scanA(0, 1)
        castA(0, 1, "act")
        a1_scan(0)
        a1_thr(0)
        fc1(0, 1)
        thr(0, 1)
        a1_mm(3)
        scanA(1, 1)
        castA(1, 1, "gp")
        a1_scan(1)
        a1_thr(1)
        fc2b1_post(0, pu2_h0)
        fc1(1, 1)
        thr(1, 1)
        scanA(2, 1)
        castA(2, 1, "gp")
        a1_scan(2)
        a1_thr(2)
        fc1(2, 1)
        thr(2, 1)
        scanA(3, 1)
        castA(3, 1, "dve")
        a1_scan(3)
        a1_thr(3)
        fc1(3, 1)
        thr(3, 1)

        # branch-2 fc2, col-tiled over samples: psum rows 32b+o, free c'
        pl2 = psum.tile([128, 1024], F32, tag="psum", name="pl2")
        for b in range(B_PER):
            for k in range(4):
                ksl = slice(k * 32, k * 32 + 32)
                nc.tensor.matmul(pl2[32 * b:32 * b + 32, 0:CP],
                                 wl2[:, ksl], l1[k][:, b * CP:(b + 1) * CP],
                                 start=(k == 0), stop=(k == 3),
                                 tile_position=(0, 32 * b),
                                 skip_group_check=True)
        nc.vector.tensor_tensor_scan(vs2[:], alpha_t[:, 0:CP], pl2[:, 0:CP],
                                     0.0, AL.mult, AL.add)
        nc.vector.tensor_tensor(o2[:], vs2[:], t2b[:], AL.is_ge)
        for b in range(B_PER):
            nc.sync.dma_start(out[b, 0:OUT_DIM, T:T + CP],
                              o2[32 * b:32 * b + OUT_DIM, :])

        # branch-1 fc2 second half (kernel tail)
        pu2_h1 = fc2b1(1)
        fc2b1_post(1, pu2_h1)


# ======================= host-side preparation =======================

def prep_core_inputs(si, sip, core):
    """Per-core data tensors, pre-packed into single-DMA SBUF layouts.
    si/sip are [32,156,2048] f32 (sip already perm-gathered)."""
    sl = si[core * B_PER:(core + 1) * B_PER]          # [4,156,2048]
    # siA [128, 4*T]: [p, b*T+t] = si[b, p, t]
    siA = np.ascontiguousarray(
        sl[:, :128, :].transpose(1, 0, 2).reshape(128, B_PER * T)
    ).astype(FP8_NP)
    siB = np.zeros((128, T), dtype=FP8_NP)
    for b in range(B_PER):
        siB[32 * b:32 * b + (C_IN - 128)] = sl[b, 128:C_IN, :]
    sp = sip[core * B_PER:(core + 1) * B_PER]         # [4,156,2048]
    # sipT [128, KT*NB2]: [p, k*NB2 + b*CP + c'] = sip[b, c', 128k+p]
    sipT = np.ascontiguousarray(
        sp.transpose(2, 0, 1).reshape(KT, 128, NB2)
        .transpose(1, 0, 2).reshape(128, KT * NB2)
    ).astype(FP8_NP)
    return {"siA": siA, "siB": siB, "sipT": sipT}


def prep_shared_inputs(W1, W2, Wl1, Wl2):
    """Weight layouts + threshold tensors, shared by all cores."""
    w1t = np.zeros((160, HID), dtype=np.float32)
    w1t[:C_IN] = W1.T
    # W1dr [128, 4*2*512]: [p, b, 0, o] = W1.T[p, o] (c 0..127);
    # [p, b, 1, o] = tail channels masked to sample b's psB rows.
    W1dr = np.zeros((128, B_PER, 2, HID), dtype=FP8_NP)
    for b in range(B_PER):
        W1dr[:, b, 0, :] = w1t[:128].astype(FP8_NP)
        W1dr[32 * b:32 * b + 32, b, 1, :] = w1t[128:160].astype(FP8_NP)
    W1dr = W1dr.reshape(128, B_PER * 2 * HID)

    # fc2 weights fp8, all k scaled 0.5 (Sign +-1 encoding), padded to 32
    # cols per k-tile. Layout [128, 4*32]: [p, k*32+o]
    w2t = W2.T.astype(np.float32)                     # [512, 20]
    W2pT = np.zeros((128, 4 * 32), dtype=FP8_NP)
    for k in range(4):
        W2pT[:, k * 32:k * 32 + OUT_DIM] = (
            0.5 * w2t[k * 128:(k + 1) * 128]).astype(FP8_NP)
    # effective (device) W2 after fp8 rounding, unscaled
    r2 = np.zeros(OUT_DIM, dtype=np.float64)
    for k in range(4):
        r2 += (W2pT[:, k * 32:k * 32 + OUT_DIM].astype(np.float64)
               .sum(axis=0)) / 0.5
    g = (1.0 - ALPHA ** (np.arange(T, dtype=np.float64) + 1)) / (1.0 - ALPHA)
    theta2 = (THETA - 0.5 * np.outer(r2, g)).astype(np.float32)   # [20, T]
    T2 = np.full((128, T), 3.0e4, dtype=np.float32)
    for b in range(B_PER):
        T2[32 * b:32 * b + OUT_DIM] = theta2
    T2 = T2.astype(BF16_NP)

    # Wl1T [128, KT*HID]: [p, k*HID+o] = Wl1[o, 128k+p]
    Wl1T = np.ascontiguousarray(
        Wl1.T.reshape(KT, 128, HID).transpose(1, 0, 2).reshape(128, KT * HID)
    ).astype(FP8_NP)

    # Wl2T [128, 4*32] bf16, 0.5-scaled (Sign +-1 l1 encoding), 32-col pad
    wl2t = Wl2.T.astype(np.float32)                   # [512, 20]
    Wl2T = np.zeros((128, 4 * 32), dtype=BF16_NP)
    for k in range(4):
        Wl2T[:, k * 32:k * 32 + OUT_DIM] = (
            0.5 * wl2t[k * 128:(k + 1) * 128]).astype(BF16_NP)
    r2l = np.zeros(OUT_DIM, dtype=np.float64)
    for k in range(4):
        r2l += (Wl2T[:, k * 32:k * 32 + OUT_DIM].astype(np.float64)
                .sum(axis=0)) / 0.5
    gcp = (1.0 - ALPHA ** (np.arange(CP, dtype=np.float64) + 1)) / (1.0 - ALPHA)
    theta2b = (THETA - 0.5 * np.outer(r2l, gcp)).astype(np.float32)  # [20,156]
    T2b = np.full((128, CP), 3.0e4, dtype=np.float32)
    for b in range(B_PER):
        T2b[32 * b:32 * b + OUT_DIM] = theta2b
    T2b = T2b.astype(BF16_NP)

    return {"W1dr": W1dr, "W2pT": W2pT, "Wl1T": Wl1T,
            "Wl2T": Wl2T, "T2": T2, "T2b": T2b}


def make_in_maps(spike_input, W1, W2, Wl1, Wl2, perm):
    si = np.asarray(spike_input, dtype=np.float32).reshape(B, C_IN, T)
    perm = np.asarray(perm).astype(np.int64)
    sip = si[:, perm, :]                              # perm-gather (layout only)
    shared = prep_shared_inputs(np.asarray(W1, np.float32),
                                np.asarray(W2, np.float32),
                                np.asarray(Wl1, np.float32),
                                np.asarray(Wl2, np.float32))
    in_maps = []
    for core in range(N_CORES):
        m = dict(shared)
        m.update(prep_core_inputs(si, sip, core))
        in_maps.append(m)
    return in_maps


_IN_SPECS = {
    "siA": ((128, B_PER * T), FP8),
    "siB": ((128, T), FP8),
    "sipT": ((128, KT * NB2), FP8),
    "W1dr": ((128, B_PER * 2 * HID), FP8),
    "W2pT": ((128, 4 * 32), FP8),
    "Wl1T": ((128, KT * HID), FP8),
    "Wl2T": ((128, 4 * 32), BF16),
    "T2": ((128, T), BF16),
    "T2b": ((128, CP), BF16),
}


def build_bass():
    nc = bacc.Bacc("TRN2", target_bir_lowering=False, debug=False)
    ins = {}
    for name, (shape, dt) in _IN_SPECS.items():
        h = nc.dram_tensor(name, list(shape), dt, kind="ExternalInput")
        ins[name] = h[:]
    out_h = nc.dram_tensor("out", [B_PER, 32, T + CP], BF16,
                           kind="ExternalOutput")
    outs = {"out": out_h[:]}
    with tile_mod.TileContext(nc) as tc:
        build_program(tc, outs, ins)
    nc.compile()
    return nc


_NC_CACHE = None


def run(inputs, trace=False, **kw):
    """Run on the 8 NeuronCores; returns (full_output, BassKernelResults)."""
    global _NC_CACHE
    if _NC_CACHE is None:
        _NC_CACHE = build_bass()
    nc = _NC_CACHE
    in_maps = make_in_maps(**inputs)
    res = run_bass_kernel_spmd(nc, in_maps, core_ids=list(range(N_CORES)),
                               trace=trace, **kw)
    parts = [res.results[c]["out"][:, :OUT_DIM, :] for c in range(N_CORES)]
    full = np.concatenate(parts, axis=0).reshape(B, OUT_DIM, 1, 1, T + CP)
    return np.ascontiguousarray(full.astype(np.float32)), res


def kernel(**inputs):
    out, _ = run(inputs)
    return out


# revision 8
# speedup vs baseline: 1.0745x; 1.0745x over previous
"""Trainium2 Bass kernel for nn_LocationSlayerRandom (SLAYER two-branch spiking net).

Contract: kernel(**inputs) takes the FULL unsharded inputs
  spike_input [32,156,1,1,2048] f32, W1 [512,156], W2 [20,512],
  Wl1 [512,2048], Wl2 [20,512], perm [156] i32
and returns the FULL output [32,20,1,1,2204] f32.

Strategy (8 cores, data-parallel over batch, 4 samples/core), v2:

Branch 1 (per sample b):  u1 = W1 @ psp_t(si)  (psp is linear => commutes)
  - psp_t(si): DVE tensor_tensor_scan along t (fp8 spike input, bf16 out),
    split into t-halves for pipelining; scans chain via initial=prev last col.
  - bf16 -> fp8 casts (DVE tensor_copy, plus two early ones on ACT) feed
  - fc1 on PE in fp8 DoubleRow: the (128-ch, 28-ch-tail) contraction pair in
    ONE 256-row pass via a step-sliced [128,2,512] AP over a packed ps8 tile.
  - thresholds all on ACT as Sign(u1-10) in {-1,0,1} fp8; fc2 weights
    pre-scaled 0.5 and the affine 0.5*rowsum(W2) folded into a host-side
    time-varying threshold T2[o,t] (bf16).
  - fc2 on PE fp8 with 4 samples in the 4 PE column groups -> one
    [128,1024] PSUM per t-half; psp scan from PSUM (bf16 out); compare vs
    T2 (all-bf16 2x DVE); bf16 output DMA.

Branch 2: ul1 = psp_c'(Wl1 @ x_tp), x_tp host-gathered/transposed (sipT).
  - A1 m-blocks on PE (fp8 DoubleRow over t), interleaved with fc1 to fill
    threshold-paced PE gaps; c'-psp scan straight from PSUM with the
    reset-pattern multiplier; l1 threshold on ACT Sign (wl2 0.5-scaled,
    rowsum correction in T2b).
  - locationFc2 col-tiled over samples: psum [32b+o, c'], so its psp scan is
    a [128,156] scan (4x shorter than [20,624]) and needs no reset pattern.

Numerics: all heavy matmuls fp8 with fp32 accumulate; psp states bf16.
The only nonlinearity is the >=10 threshold; true layer-2 potentials sit
below 3.2 (branch 1) / 2.0 (branch 2) against a threshold of 10, so
near-threshold layer-1 bit flips from low-precision weights/activations
cannot flip any output bit.
"""

from contextlib import ExitStack

import numpy as np
import ml_dtypes

import concourse.bass as bass
import concourse.mybir as mybir
from concourse import bacc
from concourse import tile as tile_mod
from concourse.bass_utils import run_bass_kernel_spmd

F32 = mybir.dt.float32
BF16 = mybir.dt.bfloat16
FP8 = mybir.dt.float8e4
AL = mybir.AluOpType
AF = mybir.ActivationFunctionType
BF16_NP = ml_dtypes.bfloat16
FP8_NP = ml_dtypes.float8_e4m3

B, C_IN, T = 32, 156, 2048
HID, OUT_DIM = 512, 20
CP = 156                      # permuted taxel axis (branch-2 "time")
N_CORES = 8
B_PER = B // N_CORES          # 4 samples per core
ALPHA = float(np.exp(-1.0 / 10.0))
THETA = 10.0
NB2 = B_PER * CP              # 624, branch-2 packed free dim
KT = T // 128                 # 16 k-tiles over t
H = T // 2                    # 1024, t-half


def build_program(tc, outs, ins):
    nc = tc.nc
    out = outs["out"]
    DR = mybir.MatmulPerfMode.DoubleRow

    with ExitStack() as ctx:
        consts = ctx.enter_context(tc.tile_pool(name="consts", bufs=1))
        work = ctx.enter_context(tc.tile_pool(name="work", bufs=1))
        psum = ctx.enter_context(tc.tile_pool(name="psum", bufs=4, space="PSUM"))

        # ---------------- constant patterns (gpsimd; SBUF only) ----------
        alpha_t = consts.tile([128, T], F32, tag="alpha")
        nc.gpsimd.memset(alpha_t[:], ALPHA)
        pat624 = consts.tile([128, NB2], F32, tag="pat624")
        nc.gpsimd.memset(pat624[:], ALPHA)
        for j in range(B_PER):
            nc.gpsimd.memset(pat624[:, j * CP:j * CP + 1], 0.0)
        bias_m10 = consts.tile([128, 1], F32, tag="bm10")
        nc.gpsimd.memset(bias_m10[:], -THETA)
        act_warm = consts.tile([128, 1], F32, tag="actwarm")
        nc.scalar.activation(act_warm[:], bias_m10[:], AF.Sign,
                             bias=bias_m10[:])

        # ---------------- inputs (ordered for earliest need) -------------
        siB = consts.tile([128, T], FP8, tag="siB")
        nc.sync.dma_start(siB[:], ins["siB"][:])
        siA = consts.tile([128, B_PER * T], FP8, tag="siA")
        nc.sync.dma_start(siA[:, 0:H], ins["siA"][:, 0:H])
        nc.sync.dma_start(siA[:, H:T], ins["siA"][:, H:T])
        w1dr = consts.tile([128, B_PER * 2 * HID], FP8, tag="w1dr")
        nc.sync.dma_start(w1dr[:], ins["W1dr"][:])
        sip = consts.tile([128, KT * NB2], FP8, tag="sip")
        nc.sync.dma_start(sip[:], ins["sipT"][:])
        wl1 = consts.tile([128, KT * HID], FP8, tag="wl1")
        nc.sync.dma_start(wl1[:], ins["Wl1T"][:])
        for b in range(1, B_PER):
            nc.sync.dma_start(siA[:, b * T:(b + 1) * T],
                              ins["siA"][:, b * T:(b + 1) * T])
        t2_t = consts.tile([128, T], BF16, tag="t2")
        nc.sync.dma_start(t2_t[:], ins["T2"][:])
        w2p = consts.tile([128, 4 * 32], FP8, tag="w2p")
        nc.sync.dma_start(w2p[:], ins["W2pT"][:])
        wl2 = consts.tile([128, 4 * 32], BF16, tag="wl2")
        nc.sync.dma_start(wl2[:], ins["Wl2T"][:])
        t2b = consts.tile([128, CP], BF16, tag="t2b")
        nc.sync.dma_start(t2b[:], ins["T2b"][:])

        # ---------------- persistent work tiles --------------------------
        psA = work.tile([128, B_PER * T], BF16, tag="psA")
        psB = work.tile([128, T], BF16, tag="psB")
        ps8 = work.tile([128, 5 * T], FP8, tag="ps8")
        ps8_3d = ps8[:].rearrange("p (n t) -> p n t", n=5)
        w1_4d = w1dr[:].rearrange("p (b k o) -> p b k o", b=B_PER, k=2)
        wl1_3d = wl1[:].rearrange("p (k o) -> p k o", o=HID)
        sip_3d = sip[:].rearrange("p (k c) -> p k c", c=NB2)
        sg = [work.tile([128, 4 * T], FP8, tag=f"sg{b}", name=f"sg{b}")
              for b in range(B_PER)]
        sg3 = [s[:].rearrange("p (m t) -> p m t", m=4) for s in sg]
        ul1 = [work.tile([128, NB2], F32, tag=f"ul1{m}", name=f"ul1{m}")
               for m in range(4)]
        l1 = [work.tile([128, NB2], BF16, tag=f"l1{m}", name=f"l1{m}")
              for m in range(4)]
        vs = work.tile([128, T], BF16, tag="vs")
        o1 = work.tile([128, T], BF16, tag="o1")
        vs2 = work.tile([128, CP], BF16, tag="vs2")
        o2 = work.tile([128, CP], BF16, tag="o2")

        # ---------------- emission helpers -------------------------------
        def scanA_q(b, q):
            sl = slice(b * T + q * 512, b * T + (q + 1) * 512)
            init = 0.0 if q == 0 else psA[:, b * T + q * 512 - 1:
                                          b * T + q * 512]
            nc.vector.tensor_tensor_scan(psA[:, sl], alpha_t[:, 0:512],
                                         siA[:, sl], init, AL.mult, AL.add)

        def scanA(b, h):
            sl = slice(b * T + h * H, b * T + (h + 1) * H)
            init = 0.0 if h == 0 else psA[:, b * T + h * H - 1:b * T + h * H]
            nc.vector.tensor_tensor_scan(psA[:, sl], alpha_t[:, 0:H],
                                         siA[:, sl], init, AL.mult, AL.add)

        def scanB_q(q):
            sl = slice(q * 512, (q + 1) * 512)
            init = 0.0 if q == 0 else psB[:, q * 512 - 1:q * 512]
            nc.vector.tensor_tensor_scan(psB[:, sl], alpha_t[:, 0:512],
                                         siB[:, sl], init, AL.mult, AL.add)

        def scanB(h):
            sl = slice(h * H, (h + 1) * H)
            init = 0.0 if h == 0 else psB[:, h * H - 1:h * H]
            nc.vector.tensor_tensor_scan(psB[:, sl], alpha_t[:, 0:H],
                                         siB[:, sl], init, AL.mult, AL.add)

        def castA_q(b, q, engine):
            sl = slice(b * T + q * 512, b * T + (q + 1) * 512)
            dst = ps8[:, b * T + q * 512:b * T + (q + 1) * 512]
            if engine == "act":
                nc.scalar.activation(dst, psA[:, sl], AF.Copy)
            else:
                nc.vector.tensor_copy(dst, psA[:, sl])

        def castB_q(q, engine):
            sl = slice(q * 512, (q + 1) * 512)
            dst = ps8[:, 4 * T + q * 512:4 * T + (q + 1) * 512]
            if engine == "act":
                nc.scalar.activation(dst, psB[:, sl], AF.Copy)
            else:
                nc.vector.tensor_copy(dst, psB[:, sl])

        def cast(block, src, h, engine):
            sl = slice(h * H, (h + 1) * H)
            dst = ps8[:, block * T + h * H:block * T + (h + 1) * H]
            if engine == "act":
                nc.scalar.activation(dst, src[:, sl], AF.Copy)
            else:
                nc.vector.tensor_copy(dst, src[:, sl])

        def castA(b, h, engine):
            sl = slice(b * T + h * H, b * T + (h + 1) * H)
            dst = ps8[:, b * T + h * H:b * T + (h + 1) * H]
            if engine == "act":
                nc.scalar.activation(dst, psA[:, sl], AF.Copy)
            elif engine == "gp":
                nc.gpsimd.tensor_copy(dst, psA[:, sl])
            else:
                nc.vector.tensor_copy(dst, psA[:, sl])

        fc1_psum = {}

        def fc1_ch(b, h, ch):
            for m in range(4):
                if (b, h, m) not in fc1_psum:
                    fc1_psum[(b, h, m)] = psum.tile(
                        [128, 1024], F32, tag="psum", name=f"pu{b}{h}{m}")
                pu = fc1_psum[(b, h, m)]
                lhs = w1_4d[:, b, :, m * 128:(m + 1) * 128]
                csl = slice(h * H + ch * 512, h * H + (ch + 1) * 512)
                rhs = ps8_3d[:, b:5:(4 - b), csl]
                nc.tensor.matmul(pu[:, ch * 512:(ch + 1) * 512], lhs, rhs,
                                 start=True, stop=True, perf_mode=DR)

        def fc1(b, h):
            # fp8 DoubleRow: (psA_b block, psB block) pair via step-sliced AP
            fc1_ch(b, h, 0)
            fc1_ch(b, h, 1)

        def thr_ch(b, h, ch):
            hs = slice(h * H + ch * 512, h * H + (ch + 1) * 512)
            for m in range(4):
                nc.scalar.activation(sg3[b][:, m, hs],
                                     fc1_psum[(b, h, m)][:, ch * 512:
                                                         (ch + 1) * 512],
                                     AF.Sign, bias=bias_m10[:])

        def thr(b, h):
            hs = slice(h * H, (h + 1) * H)
            for m in range(4):
                nc.scalar.activation(sg3[b][:, m, hs], fc1_psum[(b, h, m)][:],
                                     AF.Sign, bias=bias_m10[:])

        a1_psum = {}

        def a1_mm(m):
            pa = psum.tile([128, 1024], F32, tag="psum", name=f"pa{m}")
            a1 = pa[:, :NB2]
            msl = slice(m * 128, (m + 1) * 128)
            for ki in range(KT // 2):
                st, sp = (ki == 0), (ki == KT // 2 - 1)
                lhs = wl1_3d[:, 2 * ki:2 * ki + 2, msl]
                nc.tensor.matmul(a1[:, 0:512], lhs,
                                 sip_3d[:, 2 * ki:2 * ki + 2, 0:512],
                                 start=st, stop=sp, perf_mode=DR)
                nc.tensor.matmul(a1[:, 512:NB2], lhs,
                                 sip_3d[:, 2 * ki:2 * ki + 2, 512:NB2],
                                 start=st, stop=sp, perf_mode=DR)
            a1_psum[m] = a1

        def a1_scan(m):
            nc.vector.tensor_tensor_scan(ul1[m][:], pat624[:], a1_psum[m],
                                         0.0, AL.mult, AL.add)

        def a1_thr(m):
            nc.scalar.activation(l1[m][:], ul1[m][:], AF.Sign,
                                 bias=bias_m10[:])

        def fc2b1(h):
            pu2 = psum.tile([128, 1024], F32, tag="psum", name=f"pu2{h}")
            for b in range(B_PER):
                for k in range(4):
                    ksl = slice(k * 32, k * 32 + 32)
                    for ch in range(2):
                        csl = slice(h * H + ch * 512, h * H + (ch + 1) * 512)
                        nc.tensor.matmul(pu2[32 * b:32 * b + 32,
                                             ch * 512:(ch + 1) * 512],
                                         w2p[:, ksl], sg3[b][:, k, csl],
                                         start=(k == 0), stop=(k == 3),
                                         tile_position=(0, 32 * b),
                                         skip_group_check=True)
            return pu2

        def fc2b1_post(h, pu2):
            hs = slice(h * H, (h + 1) * H)
            init = 0.0 if h == 0 else vs[:, H - 1:H]
            nc.vector.tensor_tensor_scan(vs[:, hs], alpha_t[:, 0:H],
                                         pu2[:], init, AL.mult, AL.add)
            nc.vector.tensor_tensor(o1[:, hs], vs[:, hs], t2_t[:, hs],
                                    AL.is_ge)
            nc.sync.dma_start(
                out[:, :, hs].rearrange("b j t -> (b j) t"), o1[:, hs])

        # ================= schedule =================
        # ---- phase h0; (b0,h0) runs at 512-col granularity (ramp fill) ----
        scanB_q(0)
        scanA_q(0, 0)
        castB_q(0, "act")
        castA_q(0, 0, "dve")
        scanB_q(1)
        fc1_ch(0, 0, 0)
        thr_ch(0, 0, 0)
        scanA_q(0, 1)
        castB_q(1, "act")
        castA_q(0, 1, "dve")
        fc1_ch(0, 0, 1)
        thr_ch(0, 0, 1)
        scanA(1, 0)
        castA(1, 0, "dve")
        fc1(1, 0)
        thr(1, 0)
        a1_mm(0)
        scanA(2, 0)
        castA(2, 0, "dve")
        fc1(2, 0)
        thr(2, 0)
        a1_mm(1)
        scanA(3, 0)
        castA(3, 0, "dve")
        fc1(3, 0)
        thr(3, 0)
        a1_mm(2)
        pu2_h0 = fc2b1(0)

        # ---- phase h1 ----
        scanB(1)
        cast(4, psB, 1, "act")
        scanA(0, 1)
        castA(0, 1, "act")
        a1_scan(0)
        a1_thr(0)
        fc1(0, 1)
        thr(0, 1)
        a1_mm(3)
        scanA(1, 1)
        castA(1, 1, "dve")
        a1_scan(1)
        a1_thr(1)
        fc1(1, 1)
        thr(1, 1)
        scanA(2, 1)
        castA(2, 1, "dve")
        a1_scan(2)
        a1_thr(2)
        fc2b1_post(0, pu2_h0)
        fc1(2, 1)
        thr(2, 1)
        scanA(3, 1)
        castA(3, 1, "dve")
        a1_scan(3)
        a1_thr(3)
        fc1_ch(3, 1, 0)
        thr_ch(3, 1, 0)
        fc1_ch(3, 1, 1)
        thr_ch(3, 1, 1)

        # branch-2 fc2, col-tiled over samples: psum rows 32b+o, free c'
        pl2 = psum.tile([128, 1024], F32, tag="psum", name="pl2")
        for b in range(B_PER):
            for k in range(4):
                ksl = slice(k * 32, k * 32 + 32)
                nc.tensor.matmul(pl2[32 * b:32 * b + 32, 0:CP],
                                 wl2[:, ksl], l1[k][:, b * CP:(b + 1) * CP],
                                 start=(k == 0), stop=(k == 3),
                                 tile_position=(0, 32 * b),
                                 skip_group_check=True)
        nc.vector.tensor_tensor_scan(vs2[:], alpha_t[:, 0:CP], pl2[:, 0:CP],
                                     0.0, AL.mult, AL.add)
        nc.vector.tensor_tensor(o2[:], vs2[:], t2b[:], AL.is_ge)
        for b in range(B_PER):
            nc.sync.dma_start(out[b, 0:OUT_DIM, T:T + CP],
                              o2[32 * b:32 * b + OUT_DIM, :])

        # branch-1 fc2 second half (kernel tail)
        pu2_h1 = fc2b1(1)
        fc2b1_post(1, pu2_h1)


# ======================= host-side preparation =======================

def prep_core_inputs(si, sip, core):
    """Per-core data tensors, pre-packed into single-DMA SBUF layouts.
    si/sip are [32,156,2048] f32 (sip already perm-gathered)."""
    sl = si[core * B_PER:(core + 1) * B_PER]          # [4,156,2048]
    # siA [128, 4*T]: [p, b*T+t] = si[b, p, t]
    siA = np.ascontiguousarray(
        sl[:, :128, :].transpose(1, 0, 2).reshape(128, B_PER * T)
    ).astype(FP8_NP)
    siB = np.zeros((128, T), dtype=FP8_NP)
    for b in range(B_PER):
        siB[32 * b:32 * b + (C_IN - 128)] = sl[b, 128:C_IN, :]
    sp = sip[core * B_PER:(core + 1) * B_PER]         # [4,156,2048]
    # sipT [128, KT*NB2]: [p, k*NB2 + b*CP + c'] = sip[b, c', 128k+p]
    sipT = np.ascontiguousarray(
        sp.transpose(2, 0, 1).reshape(KT, 128, NB2)
        .transpose(1, 0, 2).reshape(128, KT * NB2)
    ).astype(FP8_NP)
    return {"siA": siA, "siB": siB, "sipT": sipT}


def prep_shared_inputs(W1, W2, Wl1, Wl2):
    """Weight layouts + threshold tensors, shared by all cores."""
    w1t = np.zeros((160, HID), dtype=np.float32)
    w1t[:C_IN] = W1.T
    # W1dr [128, 4*2*512]: [p, b, 0, o] = W1.T[p, o] (c 0..127);
    # [p, b, 1, o] = tail channels masked to sample b's psB rows.
    W1dr = np.zeros((128, B_PER, 2, HID), dtype=FP8_NP)
    for b in range(B_PER):
        W1dr[:, b, 0, :] = w1t[:128].astype(FP8_NP)
        W1dr[32 * b:32 * b + 32, b, 1, :] = w1t[128:160].astype(FP8_NP)
    W1dr = W1dr.reshape(128, B_PER * 2 * HID)

    # fc2 weights fp8, all k scaled 0.5 (Sign +-1 encoding), padded to 32
    # cols per k-tile. Layout [128, 4*32]: [p, k*32+o]
    w2t = W2.T.astype(np.float32)                     # [512, 20]
    W2pT = np.zeros((128, 4 * 32), dtype=FP8_NP)
    for k in range(4):
        W2pT[:, k * 32:k * 32 + OUT_DIM] = (
            0.5 * w2t[k * 128:(k + 1) * 128]).astype(FP8_NP)
    # effective (device) W2 after fp8 rounding, unscaled
    r2 = np.zeros(OUT_DIM, dtype=np.float64)
    for k in range(4):
        r2 += (W2pT[:, k * 32:k * 32 + OUT_DIM].astype(np.float64)
               .sum(axis=0)) / 0.5
    g = (1.0 - ALPHA ** (np.arange(T, dtype=np.float64) + 1)) / (1.0 - ALPHA)
    theta2 = (THETA - 0.5 * np.outer(r2, g)).astype(np.float32)   # [20, T]
    T2 = np.full((128, T), 3.0e4, dtype=np.float32)
    for b in range(B_PER):
        T2[32 * b:32 * b + OUT_DIM] = theta2
    T2 = T2.astype(BF16_NP)

    # Wl1T [128, KT*HID]: [p, k*HID+o] = Wl1[o, 128k+p]
    Wl1T = np.ascontiguousarray(
        Wl1.T.reshape(KT, 128, HID).transpose(1, 0, 2).reshape(128, KT * HID)
    ).astype(FP8_NP)

    # Wl2T [128, 4*32] bf16, 0.5-scaled (Sign +-1 l1 encoding), 32-col pad
    wl2t = Wl2.T.astype(np.float32)                   # [512, 20]
    Wl2T = np.zeros((128, 4 * 32), dtype=BF16_NP)
    for k in range(4):
        Wl2T[:, k * 32:k * 32 + OUT_DIM] = (
            0.5 * wl2t[k * 128:(k + 1) * 128]).astype(BF16_NP)
    r2l = np.zeros(OUT_DIM, dtype=np.float64)
    for k in range(4):
        r2l += (Wl2T[:, k * 32:k * 32 + OUT_DIM].astype(np.float64)
                .sum(axis=0)) / 0.5
    gcp = (1.0 - ALPHA ** (np.arange(CP, dtype=np.float64) + 1)) / (1.0 - ALPHA)
    theta2b = (THETA - 0.5 * np.outer(r2l, gcp)).astype(np.float32)  # [20,156]
    T2b = np.full((128, CP), 3.0e4, dtype=np.float32)
    for b in range(B_PER):
        T2b[32 * b:32 * b + OUT_DIM] = theta2b
    T2b = T2b.astype(BF16_NP)

    return {"W1dr": W1dr, "W2pT": W2pT, "Wl1T": Wl1T,
            "Wl2T": Wl2T, "T2": T2, "T2b": T2b}


def make_in_maps(spike_input, W1, W2, Wl1, Wl2, perm):
    si = np.asarray(spike_input, dtype=np.float32).reshape(B, C_IN, T)
    perm = np.asarray(perm).astype(np.int64)
    sip = si[:, perm, :]                              # perm-gather (layout only)
    shared = prep_shared_inputs(np.asarray(W1, np.float32),
                                np.asarray(W2, np.float32),
                                np.asarray(Wl1, np.float32),
                                np.asarray(Wl2, np.float32))
    in_maps = []
    for core in range(N_CORES):
        m = dict(shared)
        m.update(prep_core_inputs(si, sip, core))
        in_maps.append(m)
    return in_maps


_IN_SPECS = {
    "siA": ((128, B_PER * T), FP8),
    "siB": ((128, T), FP8),
    "sipT": ((128, KT * NB2), FP8),
    "W1dr": ((128, B_PER * 2 * HID), FP8),
    "W2pT": ((128, 4 * 32), FP8),
    "Wl1T": ((128, KT * HID), FP8),
    "Wl2T": ((128, 4 * 32), BF16),
    "T2": ((128, T), BF16),
    "T2b": ((128, CP), BF16),
}


def build_bass():
    nc = bacc.Bacc("TRN2", target_bir_lowering=False, debug=False)
    ins = {}
    for name, (shape, dt) in _IN_SPECS.items():
        h = nc.dram_tensor(name, list(shape), dt, kind="ExternalInput")
        ins[name] = h[:]
    out_h = nc.dram_tensor("out", [B_PER, 32, T + CP], BF16,
                           kind="ExternalOutput")
    outs = {"out": out_h[:]}
    with tile_mod.TileContext(nc) as tc:
        build_program(tc, outs, ins)
    nc.compile()
    return nc


_NC_CACHE = None


def run(inputs, trace=False, **kw):
    """Run on the 8 NeuronCores; returns (full_output, BassKernelResults)."""
    global _NC_CACHE
    if _NC_CACHE is None:
        _NC_CACHE = build_bass()
    nc = _NC_CACHE
    in_maps = make_in_maps(**inputs)
    res = run_bass_kernel_spmd(nc, in_maps, core_ids=list(range(N_CORES)),
                               trace=trace, **kw)
    parts = [res.results[c]["out"][:, :OUT_DIM, :] for c in range(N_CORES)]
    full = np.concatenate(parts, axis=0).reshape(B, OUT_DIM, 1, 1, T + CP)
    return np.ascontiguousarray(full.astype(np.float32)), res


def kernel(**inputs):
    out, _ = run(inputs)
    return out
